# revision 5
# baseline (speedup 1.0000x reference)
"""MoE (Deberta-style) top-2 routed SwiGLU FFN on 8 Trainium2 NeuronCores.

Expert-parallel sharding: the router (x @ Wr + noise -> top-2 -> softmax
gates) is tiny (<0.01% of FLOPs) and runs on host; tokens are dispatched
to the core owning their expert, each core runs its expert's dense SwiGLU
FFN (bf16 matmuls, f32 accumulate) over its gathered tokens, and the host
scatter-adds the gated per-expert outputs.

Self-contained: hardcodes B=4096, H=1024, I=4096, E=8, TOP_K=2.
"""

import numpy as np
import ml_dtypes

B, H, I, E = 4096, 1024, 4096, 8
TOP_K = 2
P = 128
NT = 384  # tokens per PSUM tile (<=512 f32 per bank)

_kernel_cache: dict = {}
TRACE = False          # set True (e.g. from test.py) to capture an NTFF profile
LAST_EXEC_NS = None    # neuron-profile exec time of the last run, if traced
LAST_TRACE = None


def _n_slices(C):
    return [(i, min(i + NT, C)) for i in range(0, C, NT)]


def _build_ffn_kernel(C):
    """Build + compile the per-core dense SwiGLU FFN kernel for C token
    columns: out[H, C] = swiglu(w_in.T @ xT + b_in) contracted with w_out,
    plus b_out. All transposed layouts (feature dim on partitions)."""
    import concourse.bacc as bacc
    import concourse.mybir as mybir
    import concourse.tile as tile

    f32 = mybir.dt.float32
    bf16 = mybir.dt.bfloat16
    AF = mybir.ActivationFunctionType

    KT1 = H // P    # 8   k-tiles of mm1 (contract over H)
    MT1 = I // P    # 32  m-tile *pairs* of mm1 (x1 half; x2 at mp+MT1)
    KT2 = I // P    # 32  k-tiles of mm2 (contract over I)
    MT2 = H // P    # 8   m-tiles of mm2
    nsl = _n_slices(C)

    nc = bacc.Bacc("TRN2", target_bir_lowering=False, debug=False, num_devices=E)
    xT = nc.declare_dram_parameter("xT", [H, C], bf16, isOutput=False)
    w_in = nc.declare_dram_parameter("w_in", [H, 2 * I], bf16, isOutput=False)
    w_out = nc.declare_dram_parameter("w_out", [I, H], bf16, isOutput=False)
    b_inT = nc.declare_dram_parameter("b_inT", [P, 2 * I // P], f32, isOutput=False)
    b_outT = nc.declare_dram_parameter("b_outT", [P, H // P], f32, isOutput=False)
    out = nc.declare_dram_parameter("out", [H, C], f32, isOutput=True)

    w_in_r = w_in.rearrange("(ko p) m -> p ko m", p=P)
    w_out_r = w_out.rearrange("(ko p) m -> p ko m", p=P)

    with tile.TileContext(nc) as tc:
        with (
            tc.tile_pool(name="const", bufs=1) as constp,
            tc.tile_pool(name="xpool", bufs=1) as xpool,
            tc.tile_pool(name="apool", bufs=1) as apool,
            tc.tile_pool(name="w1pool", bufs=3) as w1pool,
            tc.tile_pool(name="w2pool", bufs=3) as w2pool,
            tc.tile_pool(name="tmp", bufs=2) as tmpp,
            tc.tile_pool(name="opool", bufs=2) as opool,
            tc.tile_pool(name="psum", bufs=1, space="PSUM") as psump,
        ):
            bi = constp.tile([P, 2 * I // P], f32, tag="bi")
            nc.sync.dma_start(bi[:], b_inT[:])
            bo = constp.tile([P, H // P], f32, tag="bo")
            nc.sync.dma_start(bo[:], b_outT[:])
            xt = xpool.tile([P, KT1, C], bf16, tag="xt")
            nc.sync.dma_start(xt[:], xT.rearrange("(ko p) c -> p ko c", p=P))
            at = apool.tile([P, KT2, C], bf16, tag="at")

            # ---- mm1 + SwiGLU: at[:, mp, :] = h1 * sigmoid(h2) ----
            for mp in range(MT1):
                w1a = w1pool.tile([P, KT1, P], bf16, tag="w1a")
                nc.sync.dma_start(w1a[:], w_in_r[:, :, mp * P:(mp + 1) * P])
                w1b = w1pool.tile([P, KT1, P], bf16, tag="w1b")
                nc.sync.dma_start(
                    w1b[:], w_in_r[:, :, (mp + MT1) * P:(mp + MT1 + 1) * P]
                )
                ps1 = [
                    psump.tile([P, n1 - n0], f32, name=f"ps1_{j}", tag=f"ps1_{j}")
                    for j, (n0, n1) in enumerate(nsl)
                ]
                ps2 = [
                    psump.tile([P, n1 - n0], f32, name=f"ps2_{j}", tag=f"ps2_{j}")
                    for j, (n0, n1) in enumerate(nsl)
                ]
                for k in range(KT1):
                    for j, (n0, n1) in enumerate(nsl):
                        nc.tensor.matmul(
                            ps1[j][:], w1a[:, k, :], xt[:, k, n0:n1],
                            start=(k == 0), stop=(k == KT1 - 1),
                        )
                for k in range(KT1):
                    for j, (n0, n1) in enumerate(nsl):
                        nc.tensor.matmul(
                            ps2[j][:], w1b[:, k, :], xt[:, k, n0:n1],
                            start=(k == 0), stop=(k == KT1 - 1),
                        )
                t1 = tmpp.tile([P, C], f32, tag="t1")
                t2 = tmpp.tile([P, C], f32, tag="t2")
                for j, (n0, n1) in enumerate(nsl):
                    nc.scalar.activation(
                        t1[:, n0:n1], ps1[j][:], AF.Identity, bias=bi[:, mp:mp + 1]
                    )
                    nc.scalar.activation(
                        t2[:, n0:n1], ps2[j][:], AF.Sigmoid,
                        bias=bi[:, mp + MT1:mp + MT1 + 1],
                    )
                nc.vector.tensor_mul(at[:, mp, :], t1[:], t2[:])

            # ---- mm2: out[mh] = w_out.T @ at + b_out ----
            for mh in range(MT2):
                w2 = w2pool.tile([P, KT2, P], bf16, tag="w2")
                nc.sync.dma_start(w2[:], w_out_r[:, :, mh * P:(mh + 1) * P])
                ps3 = [
                    psump.tile([P, n1 - n0], f32, name=f"ps1_{j}", tag=f"ps1_{j}")
                    for j, (n0, n1) in enumerate(nsl)
                ]
                for k in range(KT2):
                    for j, (n0, n1) in enumerate(nsl):
                        nc.tensor.matmul(
                            ps3[j][:], w2[:, k, :], at[:, k, n0:n1],
                            start=(k == 0), stop=(k == KT2 - 1),
                        )
                ot = opool.tile([P, C], f32, tag="ot")
                for j, (n0, n1) in enumerate(nsl):
                    nc.scalar.activation(
                        ot[:, n0:n1], ps3[j][:], AF.Identity, bias=bo[:, mh:mh + 1]
                    )
                nc.sync.dma_start(out[mh * P:(mh + 1) * P, :], ot[:])

    nc.compile()
    return nc


def _route(x, noise, Wr, br):
    """Host router: returns per-token top-2 expert ids and softmax gates,
    matching jax.lax.top_k semantics (ties -> lower index first)."""
    logits = x @ Wr + br                      # [B, E] f32
    y = logits + noise
    # stable argsort of -y: descending, ties broken by lower index
    order = np.argsort(-y, axis=1, kind="stable")[:, :TOP_K]   # [B, 2]
    vals = np.take_along_axis(y, order, axis=1)                # [B, 2]
    vmax = vals.max(axis=1, keepdims=True)
    ev = np.exp(vals - vmax)
    w = ev / ev.sum(axis=1, keepdims=True)                     # [B, 2]
    return order, w.astype(np.float32)


def kernel(x, noise, Wr, br, W_in, b_in, W_out, b_out):
    from concourse.bass_utils import run_bass_kernel_spmd

    x = np.asarray(x, dtype=np.float32)
    noise = np.asarray(noise, dtype=np.float32)
    Wr = np.asarray(Wr, dtype=np.float32)
    br = np.asarray(br, dtype=np.float32)
    W_in = np.asarray(W_in, dtype=np.float32)
    b_in = np.asarray(b_in, dtype=np.float32)
    W_out = np.asarray(W_out, dtype=np.float32)
    b_out = np.asarray(b_out, dtype=np.float32)

    idxs, w = _route(x, noise, Wr, br)

    # Dispatch: token rows routed to each expert (a token can hit 2 experts).
    rows = [np.where((idxs == e).any(axis=1))[0] for e in range(E)]
    gates = [
        w[rows[e], :][idxs[rows[e]] == e].astype(np.float32) for e in range(E)
    ]
    counts = [len(r) for r in rows]
    C = max(512, -(-max(counts) // P) * P)  # capacity: round up to 128

    key = C
    if key not in _kernel_cache:
        _kernel_cache[key] = _build_ffn_kernel(C)
    nc = _kernel_cache[key]

    x_bf = x.astype(ml_dtypes.bfloat16)
    in_maps = []
    for e in range(E):
        xT_e = np.zeros((H, C), dtype=ml_dtypes.bfloat16)
        xT_e[:, :counts[e]] = x_bf[rows[e]].T
        in_maps.append({
            "xT": xT_e,
            "w_in": W_in[e].astype(ml_dtypes.bfloat16),
            "w_out": W_out[e].astype(ml_dtypes.bfloat16),
            "b_inT": np.ascontiguousarray(b_in[e].reshape(2 * I // P, P).T),
            "b_outT": np.ascontiguousarray(b_out[e].reshape(H // P, P).T),
        })

    res = run_bass_kernel_spmd(nc, in_maps, list(range(E)), trace=TRACE)
    global LAST_EXEC_NS
    LAST_EXEC_NS = res.exec_time_ns
    global LAST_TRACE
    LAST_TRACE = res.instructions_and_trace

    out = np.zeros((B, H), dtype=np.float32)
    for e in range(E):
        o = np.asarray(res.results[e]["out"], dtype=np.float32)  # [H, C]
        np.add.at(out, rows[e], gates[e][:, None] * o.T[:counts[e]])
    return out


# revision 22
# speedup vs baseline: 1.1054x; 1.1054x over previous
"""MoE (Deberta-style) top-2 routed SwiGLU FFN on 8 Trainium2 NeuronCores.

Expert-parallel with channel-split pairing: the router (x @ Wr + noise ->
top-2 -> softmax gates, <0.01% of FLOPs) runs on host. Experts are sorted
by routed-token count and paired big-with-small; each pair gets two cores,
and each core computes HALF of the intermediate (I) channels of BOTH
experts in its pair. This balances per-core work at (C_big + C_small)/2
columns instead of max(counts). The host scatter-adds the two partial
outputs per expert, scaled by the gates.

Device kernel per core, per expert section: hT = w1.T @ xT (bf16, f32
accumulate), SwiGLU fused into PSUM eviction (DVE bias-add + ACT sigmoid),
oT_partial = w2.T @ aT (+ b_out on even cores only, via input data).

Self-contained: hardcodes B=4096, H=1024, I=4096, E=8, TOP_K=2.
"""

import numpy as np
import ml_dtypes

B, H, I, E = 4096, 1024, 4096, 8
TOP_K = 2
P = 128
NT = 384        # max tokens per PSUM tile (<=512 f32 per bank)
MT1H = I // P // 2   # 16 m-tile pairs per half (x1 half; x2 at +MT1H)
KT1 = H // P         # 8  k-tiles of mm1
KT2H = I // P // 2   # 16 k-tiles of mm2 per half
MT2 = H // P         # 8  m-tiles of mm2

_kernel_cache: dict = {}
TRACE = False          # set True (e.g. from test.py) to capture an NTFF profile
LAST_EXEC_NS = None    # neuron-profile exec time of the last run, if traced
LAST_TRACE = None


def _n_slices(C):
    """Split C token columns into near-equal chunks of <=NT (PSUM bank limit)."""
    n = -(-C // NT)
    size = -(-C // n)
    out = []
    i = 0
    while i < C:
        out.append((i, min(i + size, C)))
        i += size
    return out


def _build_ffn_kernel(C1, C2):
    """Per-core kernel: two half-expert SwiGLU FFN sections (C1 and C2
    token columns), bf16 matmuls with f32 accumulation."""
    import concourse.bacc as bacc
    import concourse.mybir as mybir
    import concourse.tile as tile

    f32 = mybir.dt.float32
    bf16 = mybir.dt.bfloat16
    AF = mybir.ActivationFunctionType

    nc = bacc.Bacc("TRN2", target_bir_lowering=False, debug=False, num_devices=E)
    params = {}
    for s, C in (("A", C1), ("B", C2)):
        params[f"x{s}"] = nc.declare_dram_parameter(
            f"x{s}", [H, C], bf16, isOutput=False
        )
        # w1: [2*MT1H, P, H] — first MT1H tiles are x1 m-tiles, rest x2
        params[f"w1{s}"] = nc.declare_dram_parameter(
            f"w1{s}", [2 * MT1H, P, H], bf16, isOutput=False
        )
        # w2: [MT2, P, I/2] — per output m-tile, k-contiguous half-I rows
        params[f"w2{s}"] = nc.declare_dram_parameter(
            f"w2{s}", [MT2, P, I // 2], bf16, isOutput=False
        )
        params[f"bi{s}"] = nc.declare_dram_parameter(
            f"bi{s}", [P, 2 * MT1H], f32, isOutput=False
        )
        params[f"bo{s}"] = nc.declare_dram_parameter(
            f"bo{s}", [P, MT2], f32, isOutput=False
        )
        params[f"out{s}"] = nc.declare_dram_parameter(
            f"out{s}", [H, C], f32, isOutput=True
        )

    with tile.TileContext(nc) as tc:
        with (
            tc.tile_pool(name="const", bufs=1) as constp,
            tc.tile_pool(name="xpool", bufs=1) as xpool,
            tc.tile_pool(name="apool", bufs=1) as apool,
            tc.tile_pool(name="w1pool", bufs=4) as w1pool,
            tc.tile_pool(name="w2pool", bufs=3) as w2pool,
            tc.tile_pool(name="tmp", bufs=2) as tmpp,
            tc.tile_pool(name="opool", bufs=2) as opool,
            tc.tile_pool(name="psum", bufs=1, space="PSUM") as psump,
        ):
            for s, C in (("A", C1), ("B", C2)):
                nsl = _n_slices(C)
                w1 = params[f"w1{s}"]
                w2 = params[f"w2{s}"]
                outp = params[f"out{s}"]

                # per-section input DMAs emitted at section start so the
                # B section's transfers queue behind A's instead of
                # competing with the startup-critical A set
                bi = constp.tile([P, 2 * MT1H], f32, name=f"bi{s}", tag=f"bi{s}")
                nc.scalar.dma_start(bi[:], params[f"bi{s}"][:])
                bo = constp.tile([P, MT2], f32, name=f"bo{s}", tag=f"bo{s}")
                nc.scalar.dma_start(bo[:], params[f"bo{s}"][:])
                xt = []
                for k in range(KT1):
                    xk = xpool.tile([P, C], bf16, name=f"x{s}_{k}", tag=f"x{s}_{k}")
                    eng = nc.scalar if k % 2 == 0 else nc.sync
                    eng.dma_start(xk[:], params[f"x{s}"][k * P:(k + 1) * P, :])
                    xt.append(xk)

                def dma_w1(mp, w1=w1):
                    wa = w1pool.tile([P, H], bf16, name="w1a", tag="w1a")
                    nc.sync.dma_start(wa[:], w1[mp])
                    wb = w1pool.tile([P, H], bf16, name="w1b", tag="w1b")
                    nc.sync.dma_start(wb[:], w1[mp + MT1H])
                    return wa, wb

                w1q = [dma_w1(0), dma_w1(1), dma_w1(2)]
                at = apool.tile([P, KT2H, C], bf16, name=f"at{s}", tag=f"at{s}")

                # ---- mm1 + SwiGLU: at[:, mp, :] = h1 * sigmoid(h2) ----
                for mp in range(MT1H):
                    w1a, w1b = w1q.pop(0)
                    if mp + 3 < MT1H:
                        w1q.append(dma_w1(mp + 3))
                    ps1 = [
                        psump.tile([P, n1 - n0], f32, name=f"ps1_{j}", tag=f"ps1_{j}")
                        for j, (n0, n1) in enumerate(nsl)
                    ]
                    ps2 = [
                        psump.tile([P, n1 - n0], f32, name=f"ps2_{j}", tag=f"ps2_{j}")
                        for j, (n0, n1) in enumerate(nsl)
                    ]
                    t1 = tmpp.tile([P, C], f32, name="t1", tag="t1")
                    t2 = tmpp.tile([P, C], f32, name="t2", tag="t2")
                    for k in range(KT1):
                        for j, (n0, n1) in enumerate(nsl):
                            nc.tensor.matmul(
                                ps1[j][:], w1a[:, k * P:(k + 1) * P],
                                xt[k][:, n0:n1],
                                start=(k == 0), stop=(k == KT1 - 1),
                            )
                    # h1 eviction on DVE (bias add) while h2 matmuls run;
                    # ACT does only Sigmoid so its LUT never reloads
                    for j, (n0, n1) in enumerate(nsl):
                        nc.vector.tensor_scalar_add(
                            t1[:, n0:n1], ps1[j][:], bi[:, mp:mp + 1]
                        )
                    for k in range(KT1):
                        for j, (n0, n1) in enumerate(nsl):
                            nc.tensor.matmul(
                                ps2[j][:], w1b[:, k * P:(k + 1) * P],
                                xt[k][:, n0:n1],
                                start=(k == 0), stop=(k == KT1 - 1),
                            )
                    for j, (n0, n1) in enumerate(nsl):
                        nc.scalar.activation(
                            t2[:, n0:n1], ps2[j][:], AF.Sigmoid,
                            bias=bi[:, MT1H + mp:MT1H + mp + 1],
                        )
                    nc.vector.tensor_mul(at[:, mp, :], t1[:], t2[:])

                # ---- mm2: out[mh] = w2.T @ at (+ b_out via input data) ----
                for mh in range(MT2):
                    w2t = w2pool.tile([P, I // 2], bf16, name="w2", tag="w2")
                    nc.sync.dma_start(w2t[:], w2[mh])
                    pst = "ps1" if mh % 2 == 0 else "ps2"
                    ps3 = [
                        psump.tile(
                            [P, n1 - n0], f32, name=f"{pst}_{j}", tag=f"{pst}_{j}"
                        )
                        for j, (n0, n1) in enumerate(nsl)
                    ]
                    for k in range(KT2H):
                        for j, (n0, n1) in enumerate(nsl):
                            nc.tensor.matmul(
                                ps3[j][:], w2t[:, k * P:(k + 1) * P],
                                at[:, k, n0:n1],
                                start=(k == 0), stop=(k == KT2H - 1),
                            )
                    ot = opool.tile([P, C], f32, name="ot", tag="ot")
                    for j, (n0, n1) in enumerate(nsl):
                        nc.vector.tensor_scalar_add(
                            ot[:, n0:n1], ps3[j][:], bo[:, mh:mh + 1]
                        )
                        nc.sync.dma_start(
                            outp[mh * P:(mh + 1) * P, n0:n1], ot[:, n0:n1]
                        )

    nc.compile()
    return nc


def _tile_w_in(W):
    """[H, 2I] -> [2I//P, P, H]: per-m-tile [P, H] contiguous blocks."""
    return np.ascontiguousarray(
        W.astype(ml_dtypes.bfloat16)
        .reshape(H // P, P, 2 * I // P, P).transpose(2, 1, 0, 3)
        .reshape(2 * I // P, P, H)
    )


def _tile_w_out(W):
    """[I/2, H] -> [H//P, P, I/2]: k-contiguous blocks per output m-tile."""
    ih = W.shape[0]
    return np.ascontiguousarray(
        W.astype(ml_dtypes.bfloat16)
        .reshape(ih // P, P, H // P, P).transpose(2, 1, 0, 3)
        .reshape(H // P, P, ih)
    )


def _route(x, noise, Wr, br):
    """Host router: per-token top-2 expert ids and softmax gates, matching
    jax.lax.top_k semantics (ties -> lower index first)."""
    logits = x @ Wr + br                      # [B, E] f32
    y = logits + noise
    order = np.argsort(-y, axis=1, kind="stable")[:, :TOP_K]   # [B, 2]
    vals = np.take_along_axis(y, order, axis=1)                # [B, 2]
    vmax = vals.max(axis=1, keepdims=True)
    ev = np.exp(vals - vmax)
    w = ev / ev.sum(axis=1, keepdims=True)                     # [B, 2]
    return order, w.astype(np.float32)


def _pad_T(xb, C):
    """Gathered token rows [n, H] (bf16) -> padded transposed [H, C]."""
    o = np.zeros((H, C), dtype=ml_dtypes.bfloat16)
    o[:, :xb.shape[0]] = xb.T
    return o


def kernel(x, noise, Wr, br, W_in, b_in, W_out, b_out):
    from concourse.bass_utils import run_bass_kernel_spmd

    x = np.asarray(x, dtype=np.float32)
    noise = np.asarray(noise, dtype=np.float32)
    Wr = np.asarray(Wr, dtype=np.float32)
    br = np.asarray(br, dtype=np.float32)
    W_in = np.asarray(W_in, dtype=np.float32)
    b_in = np.asarray(b_in, dtype=np.float32)
    W_out = np.asarray(W_out, dtype=np.float32)
    b_out = np.asarray(b_out, dtype=np.float32)

    idxs, w = _route(x, noise, Wr, br)
    rows = [np.where((idxs == e).any(axis=1))[0] for e in range(E)]
    gates = [
        w[rows[e], :][idxs[rows[e]] == e].astype(np.float32) for e in range(E)
    ]
    counts = np.array([len(r) for r in rows])

    # pair largest with smallest; pair i -> cores 2i (channels [0, I/2))
    # and 2i+1 (channels [I/2, I))
    order = np.argsort(-counts, kind="stable")
    bigs, smalls = order[:E // 2], order[E // 2:][::-1]
    C1 = max(512, int(counts[bigs].max()))
    C2 = max(512, int(counts[smalls].max()))

    key = (C1, C2)
    if key not in _kernel_cache:
        _kernel_cache[key] = _build_ffn_kernel(C1, C2)
    nc = _kernel_cache[key]

    x_bf = x.astype(ml_dtypes.bfloat16)
    w1t = {}   # expert -> [64, P, H] tiled w_in (shared by its two cores)
    in_maps = []
    for i in range(E // 2):
        eA, eB = int(bigs[i]), int(smalls[i])
        for e in (eA, eB):
            if e not in w1t:
                w1t[e] = _tile_w_in(W_in[e])
        xA = _pad_T(x_bf[rows[eA]], C1)
        xB = _pad_T(x_bf[rows[eB]], C2)
        for h in (0, 1):
            m = {}
            for s, e, xp in (("A", eA, xA), ("B", eB, xB)):
                sel = np.r_[h * MT1H:(h + 1) * MT1H]
                m[f"x{s}"] = xp
                m[f"w1{s}"] = np.ascontiguousarray(
                    np.concatenate([w1t[e][sel], w1t[e][2 * MT1H + sel]])
                )
                m[f"w2{s}"] = _tile_w_out(
                    W_out[e][h * (I // 2):(h + 1) * (I // 2)]
                )
                bsel = np.concatenate([sel, 2 * MT1H + sel])
                m[f"bi{s}"] = np.ascontiguousarray(
                    b_in[e].reshape(2 * I // P, P).T[:, bsel]
                )
                m[f"bo{s}"] = (
                    np.ascontiguousarray(b_out[e].reshape(MT2, P).T)
                    if h == 0 else np.zeros((P, MT2), np.float32)
                )
            in_maps.append(m)

    global LAST_EXEC_NS, LAST_TRACE
    for attempt in range(3):
        res = run_bass_kernel_spmd(nc, in_maps, list(range(E)), trace=TRACE)
        LAST_EXEC_NS = res.exec_time_ns
        LAST_TRACE = res.instructions_and_trace

        out = np.zeros((B, H), dtype=np.float32)
        for i in range(E // 2):
            eA, eB = int(bigs[i]), int(smalls[i])
            for s, e in (("A", eA), ("B", eB)):
                o = (np.asarray(res.results[2 * i][f"out{s}"], dtype=np.float32)
                     + np.asarray(res.results[2 * i + 1][f"out{s}"],
                                  dtype=np.float32))
                np.add.at(out, rows[e], gates[e][:, None] * o.T[:len(rows[e])])
        # inputs are O(1)-scaled; nonfinite output means a transient
        # device/transport fault — retry the execution
        if np.isfinite(out).all():
            return out
    return out
